# revision 13
# baseline (speedup 1.0000x reference)
"""AFPM (adaptive per-patch modulation) kernel for 8 TRN2 NeuronCores.

Reference computation (B=8, C=64, H=W=512, K=8, HID=64):
  - d[l]: normalized distance of each 8x8 patch center from image center
  - pk[l, kk] / pb[l]: tiny MLPs of d (host-precomputable, data-independent)
  - feats[b,c,l] = sum_kk patches[b,c,kk,l] * pk[l,kk] + pb[l]
  - feats2 = conv_w @ feats + conv_b           (1x1 conv over channels)
  - out patches = patches * feats2[:, :, None, :]

Sharding: core i handles patch-rows i*8..i*8+7 for ALL 8 images.

v6: unit = (patch-row t, image-pair v), t-major; 32 identical units of
[128 part = (u2, c64), free 4096 = (dy8, pw64, dx8)], 1 MiB DMA each
direction.  HBM floor is 64 MiB / 358 GB/s ~= 187 us; every engine is
budgeted under that:

  DMA  in   : 1 MiB, rings alternate sync/scalar per unit
  DVE  mul  : PROD = xb * PKREP[t]      flat TT, 2x bf16   (~2.2us)
  DVE  L1   : dy 8->4 halving add                          (~1.2us)
  Pool L2a  : dy 4->2   (gpsimd; runs ~1.5x slow under DVE port
  Pool L2b  : dy 2->1    contention -- only these 2 stages fit)
  PE   g    : g = sum_dx bd.T @ t3[:,:,dx]  (+) w2.T @ [pb;1]
              8 accumulating strided matmuls fold the dx-reduce into
              the conv matmul, all in PSUM f32
  ACT  gexp : bf16(g) expanded over dx
  DVE  out  : OUT(prod buf) = xb * bcast(gexp), deferred 3 units to
              hide the Pool->PE->ACT gexp latency
  DMA  out  : 1 MiB from the prod buffer on the opposite ring

pk tables ship UNREPLICATED ([8,1,4096] bf16, 64 KiB total vs 8 MiB
host-replicated in v4) and are broadcast across partitions on-device:
PE ones-matmul -> PSUM -> copy to SBUF (DVE for row 0 during pipeline
fill, ACT for the rest), one row ahead of use.
"""

import math
import sys

import numpy as np

for _p in ("/opt/trn_rl_repo",):
    if _p not in sys.path:
        sys.path.insert(0, _p)

import concourse.bass as bass
import concourse.tile as tile
from concourse import bacc, mybir
from concourse.bass_utils import run_bass_kernel_spmd

B, C, H, W, K, HID = 8, 64, 512, 512, 8, 64
NH, NW = H // K, W // K          # 64, 64
L = NH * NW                      # 4096
NR = 8                           # patch-rows per core
NV = 4                           # image-pairs (u=2 images on partitions)
FD = K * W                       # 4096 free dim per unit
F32 = mybir.dt.float32
BF16 = mybir.dt.bfloat16

_ERF = np.frompyfunc(math.erf, 1, 1)


def _gelu(x):
    x = np.asarray(x, np.float64)
    return 0.5 * x * (1.0 + _ERF(x / math.sqrt(2.0)).astype(np.float64))


def _host_tables(w1k, b1k, w2k, b2k, w1b, b1b, w2b, b2b, conv_w, conv_b):
    """pk/pb via the tiny MLPs; packed as PKR [NH, FD] plus fold consts."""
    cy = cx = H / 2.0
    max_d = math.sqrt(cy * cy + cx * cx)
    py = np.arange(NH, dtype=np.float64) * K + K / 2.0
    px = np.arange(NW, dtype=np.float64) * K + K / 2.0
    d = np.sqrt((py - cy)[:, None] ** 2 + (px - cx)[None, :] ** 2) / max_d
    d = d.reshape(L, 1)

    pk = _gelu(d @ w1k.astype(np.float64) + b1k) @ w2k.astype(np.float64) + b2k
    pb = (_gelu(d @ w1b.astype(np.float64) + b1b) @ w2b.astype(np.float64) + b2b)[:, 0]

    import ml_dtypes

    # PKR[ph, dy*W + pw*K + dx] = pk[ph*NW + pw, dy*K + dx]  (bf16 on device)
    pkr = (
        pk.reshape(NH, NW, K, K).transpose(0, 2, 1, 3).reshape(NH, FD)
    ).astype(ml_dtypes.bfloat16)

    # g = sum_dx bd.T @ t3_dx  +  w2.T @ [pb_row; 1]
    cw1 = conv_w.astype(np.float64).sum(axis=1)
    w2 = np.stack([np.tile(cw1, 2), np.tile(conv_b.astype(np.float64), 2)]).astype(
        np.float32
    )  # [2, 128]

    bd = np.zeros((128, 128), np.float32)
    bd[0:C, 0:C] = conv_w.T
    bd[C:128, C:128] = conv_w.T
    return pkr, pb, w2, bd, pk


def build_program():
    nc = bacc.Bacc("TRN2", target_bir_lowering=False, debug=False, num_devices=8)
    x_d = nc.dram_tensor("x", [NV, 128, NR * K, W], BF16, kind="ExternalInput")
    pkr_d = nc.dram_tensor("pkr", [NR, 1, FD], BF16, kind="ExternalInput")
    pkrep0_d = nc.dram_tensor("pkrep0", [128, FD], BF16, kind="ExternalInput")
    pbx_d = nc.dram_tensor("pbx", [2, NR * NW], BF16, kind="ExternalInput")
    w2_d = nc.dram_tensor("w2", [2, 128], BF16, kind="ExternalInput")
    bd_d = nc.dram_tensor("bd", [128, 128], BF16, kind="ExternalInput")
    out_d = nc.dram_tensor("out", [NV, 128, NR * K, W], BF16, kind="ExternalOutput")

    # [t, v, p=(u c), dy, w] views of the DRAM image slices
    xr = x_d.ap().rearrange("v p (t dy) w -> t v p dy w", dy=K)
    outr = out_d.ap().rearrange("v p (t dy) w -> t v p dy w", dy=K)

    with tile.TileContext(nc) as tc:
        with (
            tc.tile_pool(name="const", bufs=1) as constp,
            tc.tile_pool(name="pkline", bufs=1) as pklinep,
            tc.tile_pool(name="pkrep", bufs=2) as pkrepp,
            tc.tile_pool(name="xbp", bufs=6) as xbp,
            tc.tile_pool(name="prodp", bufs=6) as prodp,
            tc.tile_pool(name="t1p", bufs=2) as t1p,
            tc.tile_pool(name="t2p", bufs=2) as t2p,
            tc.tile_pool(name="t3p", bufs=3) as t3p,
            tc.tile_pool(name="gexpp", bufs=4) as gexpp,
            tc.tile_pool(name="gpsum", bufs=4, space="PSUM") as gpsum,
            tc.tile_pool(name="pkpsum", bufs=2, space="PSUM") as pkpsum,
        ):
            # ones row for the PE partition-broadcast of pk lines
            ones_t = constp.tile([1, 128], BF16)
            nc.vector.memset(ones_t[:], 1.0)

            # small consts on the scalar HWDGE ring (sync ring carries the
            # first x chunk; gpsimd SWDGE carries the pk lines)
            pbx = constp.tile([2, NR * NW], BF16)
            nc.scalar.dma_start(pbx[:], pbx_d[:])
            w2t = constp.tile([2, 128], BF16)
            nc.scalar.dma_start(w2t[:], w2_d[:])
            bdt = constp.tile([128, 128], BF16)
            nc.scalar.dma_start(bdt[:], bd_d[:])

            def in_ring(i):
                return nc.sync if i % 2 == 0 else nc.scalar

            def out_ring(i):
                return nc.scalar if i % 2 == 0 else nc.sync

            def build_pkrep(t):
                """Replicate pk row t across all 128 partitions: HWDGE line
                load -> PE ones-matmul -> PSUM -> ACT copy to SBUF."""
                pkrep = pkrepp.tile([128, FD], BF16)
                pkl = pklinep.tile([1, FD], BF16)
                (nc.sync if t % 2 == 0 else nc.scalar).dma_start(pkl[:], pkr_d[t])
                for ch in range(FD // 512):
                    ps = pkpsum.tile([128, 512], F32)
                    nc.tensor.matmul(
                        ps[:],
                        ones_t[:],
                        pkl[:, ch * 512 : (ch + 1) * 512],
                        start=True,
                        stop=True,
                    )
                    nc.scalar.copy(pkrep[:, ch * 512 : (ch + 1) * 512], ps[:])
                return pkrep

            def emit_outmul(st):
                """Deferred modulation+store, three units behind: the unit's
                dead prod buffer becomes the output buffer.  NOTE: 4-D APs
                (leading singleton) — the 3-D form of this broadcast drops
                DVE to 1x mode on HW; the 4-D form (as in v4) keeps 2x."""
                prod, xb, gexp, t, v, i, w0, wlen = st
                o4 = prod.rearrange("p (tl dy q) -> p tl dy q", tl=1, dy=K)
                x4 = xb.rearrange("p (tl dy q) -> p tl dy q", tl=1, dy=K)
                g4 = gexp.rearrange("p (tl a q) -> p tl a q", tl=1, a=1)
                x4b, g4b = bass.broadcast_tensor_aps(x4, g4)
                nc.vector.tensor_tensor(o4, x4b, g4b, op=mybir.AluOpType.mult)
                out_ring(i).dma_start(
                    outr[t, v][:, :, w0 : w0 + wlen],
                    prod.rearrange("p (dy w) -> p dy w", dy=K),
                )

            pkreps = {}
            # row 0 ships host-replicated (1 MiB on the otherwise-idle
            # scalar ring at t=0) so the first mul isn't gated on the
            # PE/ACT broadcast ping-pong; rows 1-7 build on-device.
            pkreps[0] = pkrepp.tile([128, FD], BF16, name="pkrep0")
            nc.scalar.dma_start(pkreps[0][:], pkrep0_d[:])

            pend = []

            def emit_unit(i, t, v, w0, wlen):
                xb = xbp.tile([128, K * wlen], BF16)
                in_ring(i).dma_start(
                    xb.rearrange("p (dy w) -> p dy w", dy=K),
                    xr[t, v][:, :, w0 : w0 + wlen],
                )

                # PROD = xb * pkrep[t]  (2x bf16)
                prod = prodp.tile([128, K * wlen], BF16)
                pkq = pkreps[t].rearrange("p (dy w) -> p dy w", dy=K)
                nc.vector.tensor_tensor(
                    prod.rearrange("p (dy w) -> p dy w", dy=K),
                    xb.rearrange("p (dy w) -> p dy w", dy=K),
                    pkq[:, :, w0 : w0 + wlen],
                    op=mybir.AluOpType.mult,
                )

                with nc.allow_low_precision("pairwise bf16 tree adds"):
                    # L1: dy 8 -> 4
                    t1 = t1p.tile([128, K * wlen // 2], BF16)
                    pr4 = prod.rearrange("p (dy q) -> p dy q", dy=K)
                    t14 = t1.rearrange("p (dy q) -> p dy q", dy=K // 2)
                    nc.vector.tensor_tensor(
                        t14,
                        pr4[:, 0 : K // 2, :],
                        pr4[:, K // 2 : K, :],
                        op=mybir.AluOpType.add,
                    )
                    # L2a: dy 4 -> 2
                    t2 = t2p.tile([128, K * wlen // 4], BF16)
                    t14b = t1.rearrange("p (dy q) -> p dy q", dy=4)
                    t24 = t2.rearrange("p (dy q) -> p dy q", dy=2)
                    nc.vector.tensor_tensor(
                        t24,
                        t14b[:, 0:2, :],
                        t14b[:, 2:4, :],
                        op=mybir.AluOpType.add,
                    )
                    # L2b: dy 2 -> 1
                    t3 = t3p.tile([128, wlen], BF16)
                    nc.vector.tensor_tensor(
                        t3.rearrange("p (a q) -> p a q", a=1),
                        t24[:, 0:1, :],
                        t24[:, 1:2, :],
                        op=mybir.AluOpType.add,
                    )

                # g = sum_dx bd.T @ t3[:, :, dx]  +  w2.T @ [pb; 1]
                # (the dx-reduce rides the conv matmul's PSUM accumulation)
                npw = wlen // K
                g = gpsum.tile([128, npw], F32)
                t3x = t3.rearrange("p (q dx) -> p q dx", dx=K)
                for j in range(K):
                    nc.tensor.matmul(
                        g[:],
                        bdt[:],
                        t3x[:, :, j : j + 1],
                        start=(j == 0),
                        stop=False,
                    )
                nc.tensor.matmul(
                    g[:],
                    w2t[:],
                    pbx[:, t * NW + w0 // K : t * NW + w0 // K + npw],
                    start=False,
                    stop=True,
                )

                # modulation of the unit THREE back
                if len(pend) == 3:
                    emit_outmul(pend.pop(0))

                # cast g to bf16 expanded over dx (dense 8-elem runs)
                gexp = gexpp.tile([128, wlen], BF16, tag="gexp")
                ge3 = gexp.rearrange("p (q dx) -> p q dx", dx=K)
                gs3 = g.rearrange("p (q a) -> p q a", a=1)
                ge3b, gs3b = bass.broadcast_tensor_aps(ge3, gs3)
                nc.scalar.copy(ge3b, gs3b)

                pend.append((prod, xb, gexp, t, v, i, w0, wlen))

            # first and last (t, v) run as quarter-width units so the
            # pipeline ramps/drains on ~0.25 MiB chains
            units = [(0, 0, q * 128, 128) for q in range(4)]
            for i in range(1, NR * NV - 1):
                t, v = divmod(i, NV)
                units.append((t, v, 0, W))
            units += [(NR - 1, NV - 1, q * 128, 128) for q in range(4)]

            for i, (t, v, w0, wlen) in enumerate(units):
                unit_idx = t * NV + v
                # build row t+1's table one row ahead of use
                if unit_idx % NV == 1 and w0 == 0 and t + 1 < NR:
                    pkreps[t + 1] = build_pkrep(t + 1)
                emit_unit(i, t, v, w0, wlen)

            for st in pend:
                emit_outmul(st)

    nc.compile()
    return nc


_PROGRAM = None
LAST_RESULT = None


def make_in_maps(x, pkr, pb, w2, bd):
    import ml_dtypes

    in_maps = []
    for i in range(8):
        r0 = i * NR
        x_core = (
            np.ascontiguousarray(x[:, :, r0 * K : (r0 + NR) * K, :])
            .astype(ml_dtypes.bfloat16)
            .reshape(NV, 128, NR * K, W)
        )
        pkrz = np.ascontiguousarray(pkr[r0 : r0 + NR]).reshape(NR, 1, FD)
        pkrep0 = np.ascontiguousarray(np.broadcast_to(pkr[r0], (128, FD)))
        pbx = np.empty((2, NR * NW), np.float32)
        pbx[0] = pb[r0 * NW : (r0 + NR) * NW]
        pbx[1] = 1.0
        pbx = pbx.astype(ml_dtypes.bfloat16)
        in_maps.append(
            {
                "x": x_core,
                "pkr": pkrz,
                "pkrep0": pkrep0,
                "pbx": pbx,
                "w2": w2.astype(ml_dtypes.bfloat16),
                "bd": bd.astype(ml_dtypes.bfloat16),
            }
        )
    return in_maps


def kernel(**inputs):
    global _PROGRAM, LAST_RESULT
    x = np.ascontiguousarray(np.asarray(inputs["x"], dtype=np.float32))
    pkr, pb, w2, bd, pk = _host_tables(
        *[
            np.asarray(inputs[k], dtype=np.float32)
            for k in (
                "w1k", "b1k", "w2k", "b2k",
                "w1b", "b1b", "w2b", "b2b",
                "conv_w", "conv_b",
            )
        ]
    )
    if _PROGRAM is None:
        _PROGRAM = build_program()
    nc = _PROGRAM

    in_maps = make_in_maps(x, pkr, pb, w2, bd)

    conv_w = np.asarray(inputs["conv_w"], np.float64)
    conv_b = np.asarray(inputs["conv_b"], np.float64)

    def _spot_check(out):
        """Verify a sample of patches against the exact host formula;
        catches the rare silent device corruption (bf16 path ~0.4%/elem)."""
        rng = np.random.default_rng(1234)
        worst = 0.0
        for _ in range(32):
            b = int(rng.integers(B))
            ph = int(rng.integers(NH))
            pw = int(rng.integers(NW))
            l = ph * NW + pw
            patch = x[b, :, ph * K : (ph + 1) * K, pw * K : (pw + 1) * K]
            patch = patch.reshape(C, K * K).astype(np.float64)
            feats = patch @ pk[l] + pb[l]
            g = conv_w @ feats + conv_b
            exp = patch * g[:, None]
            got = out[b, :, ph * K : (ph + 1) * K, pw * K : (pw + 1) * K]
            got = got.reshape(C, K * K).astype(np.float64)
            denom = np.linalg.norm(exp) + 1e-30
            worst = max(worst, float(np.linalg.norm(got - exp) / denom))
        return worst

    res = None
    for attempt in range(4):
        try:
            res = run_bass_kernel_spmd(nc, in_maps, list(range(8)))
        except Exception:
            if attempt == 3:
                raise
            continue
        out = np.empty((B, C, H, W), np.float32)
        for i in range(8):
            r0 = i * NR
            out[:, :, r0 * K : (r0 + NR) * K, :] = (
                res.results[i]["out"].astype(np.float32).reshape(B, C, NR * K, W)
            )
        err = _spot_check(out)
        if err < 0.05:
            break
        if attempt == 3:
            raise RuntimeError(f"device output failed spot check ({err:.3f})")
    LAST_RESULT = res
    return out


# revision 14
# speedup vs baseline: 1.0512x; 1.0512x over previous
"""AFPM (adaptive per-patch modulation) kernel for 8 TRN2 NeuronCores.

Reference computation (B=8, C=64, H=W=512, K=8, HID=64):
  - d[l]: normalized distance of each 8x8 patch center from image center
  - pk[l, kk] / pb[l]: tiny MLPs of d (host-precomputable, data-independent)
  - feats[b,c,l] = sum_kk patches[b,c,kk,l] * pk[l,kk] + pb[l]
  - feats2 = conv_w @ feats + conv_b           (1x1 conv over channels)
  - out patches = patches * feats2[:, :, None, :]

Sharding: core i handles patch-rows i*8..i*8+7 for ALL 8 images.

v6: unit = (patch-row t, image-pair v), t-major; 32 identical units of
[128 part = (u2, c64), free 4096 = (dy8, pw64, dx8)], 1 MiB DMA each
direction.  HBM floor is 64 MiB / 358 GB/s ~= 187 us; every engine is
budgeted under that:

  DMA  in   : 1 MiB, rings alternate sync/scalar per unit
  DVE  mul  : PROD = xb * PKREP[t]      flat TT, 2x bf16   (~2.2us)
  DVE  L1   : dy 8->4 halving add                          (~1.2us)
  Pool L2a  : dy 4->2   (gpsimd; runs ~1.5x slow under DVE port
  Pool L2b  : dy 2->1    contention -- only these 2 stages fit)
  PE   g    : g = sum_dx bd.T @ t3[:,:,dx]  (+) w2.T @ [pb;1]
              8 accumulating strided matmuls fold the dx-reduce into
              the conv matmul, all in PSUM f32
  ACT  gexp : bf16(g) expanded over dx
  DVE  out  : OUT(prod buf) = xb * bcast(gexp), deferred 3 units to
              hide the Pool->PE->ACT gexp latency
  DMA  out  : 1 MiB from the prod buffer on the opposite ring

pk tables ship UNREPLICATED ([8,1,4096] bf16, 64 KiB total vs 8 MiB
host-replicated in v4) and are broadcast across partitions on-device:
PE ones-matmul -> PSUM -> copy to SBUF (DVE for row 0 during pipeline
fill, ACT for the rest), one row ahead of use.
"""

import math
import sys

import numpy as np

for _p in ("/opt/trn_rl_repo",):
    if _p not in sys.path:
        sys.path.insert(0, _p)

import concourse.bass as bass
import concourse.tile as tile
from concourse import bacc, mybir
from concourse.bass_utils import run_bass_kernel_spmd

B, C, H, W, K, HID = 8, 64, 512, 512, 8, 64
NH, NW = H // K, W // K          # 64, 64
L = NH * NW                      # 4096
NR = 8                           # patch-rows per core
NV = 4                           # image-pairs (u=2 images on partitions)
FD = K * W                       # 4096 free dim per unit
F32 = mybir.dt.float32
BF16 = mybir.dt.bfloat16

_ERF = np.frompyfunc(math.erf, 1, 1)


def _gelu(x):
    x = np.asarray(x, np.float64)
    return 0.5 * x * (1.0 + _ERF(x / math.sqrt(2.0)).astype(np.float64))


def _host_tables(w1k, b1k, w2k, b2k, w1b, b1b, w2b, b2b, conv_w, conv_b):
    """pk/pb via the tiny MLPs; packed as PKR [NH, FD] plus fold consts."""
    cy = cx = H / 2.0
    max_d = math.sqrt(cy * cy + cx * cx)
    py = np.arange(NH, dtype=np.float64) * K + K / 2.0
    px = np.arange(NW, dtype=np.float64) * K + K / 2.0
    d = np.sqrt((py - cy)[:, None] ** 2 + (px - cx)[None, :] ** 2) / max_d
    d = d.reshape(L, 1)

    pk = _gelu(d @ w1k.astype(np.float64) + b1k) @ w2k.astype(np.float64) + b2k
    pb = (_gelu(d @ w1b.astype(np.float64) + b1b) @ w2b.astype(np.float64) + b2b)[:, 0]

    import ml_dtypes

    # PKR[ph, dy*W + pw*K + dx] = pk[ph*NW + pw, dy*K + dx]  (bf16 on device)
    pkr = (
        pk.reshape(NH, NW, K, K).transpose(0, 2, 1, 3).reshape(NH, FD)
    ).astype(ml_dtypes.bfloat16)

    # g = sum_dx bd.T @ t3_dx  +  w2.T @ [pb_row; 1]
    cw1 = conv_w.astype(np.float64).sum(axis=1)
    w2 = np.stack([np.tile(cw1, 2), np.tile(conv_b.astype(np.float64), 2)]).astype(
        np.float32
    )  # [2, 128]

    bd = np.zeros((128, 128), np.float32)
    bd[0:C, 0:C] = conv_w.T
    bd[C:128, C:128] = conv_w.T
    return pkr, pb, w2, bd, pk


def build_program():
    nc = bacc.Bacc("TRN2", target_bir_lowering=False, debug=False, num_devices=8)
    x_d = nc.dram_tensor("x", [NV, 128, NR * K, W], BF16, kind="ExternalInput")
    pkr_d = nc.dram_tensor("pkr", [NR, 1, FD], BF16, kind="ExternalInput")
    pkrep0_d = nc.dram_tensor("pkrep0", [128, FD], BF16, kind="ExternalInput")
    pbx_d = nc.dram_tensor("pbx", [2, NR * NW], BF16, kind="ExternalInput")
    w2_d = nc.dram_tensor("w2", [2, 128], BF16, kind="ExternalInput")
    bd_d = nc.dram_tensor("bd", [128, 128], BF16, kind="ExternalInput")
    out_d = nc.dram_tensor("out", [NV, 128, NR * K, W], BF16, kind="ExternalOutput")

    # [t, v, p=(u c), dy, w] views of the DRAM image slices
    xr = x_d.ap().rearrange("v p (t dy) w -> t v p dy w", dy=K)
    outr = out_d.ap().rearrange("v p (t dy) w -> t v p dy w", dy=K)

    with tile.TileContext(nc) as tc:
        with (
            tc.tile_pool(name="const", bufs=1) as constp,
            tc.tile_pool(name="pkline", bufs=1) as pklinep,
            tc.tile_pool(name="pkrep", bufs=2) as pkrepp,
            tc.tile_pool(name="xbp", bufs=6) as xbp,
            tc.tile_pool(name="prodp", bufs=6) as prodp,
            tc.tile_pool(name="t1p", bufs=2) as t1p,
            tc.tile_pool(name="t2p", bufs=2) as t2p,
            tc.tile_pool(name="t3p", bufs=3) as t3p,
            tc.tile_pool(name="gexpp", bufs=4) as gexpp,
            tc.tile_pool(name="gpsum", bufs=4, space="PSUM") as gpsum,
            tc.tile_pool(name="pkpsum", bufs=2, space="PSUM") as pkpsum,
        ):
            # ones row for the PE partition-broadcast of pk lines
            ones_t = constp.tile([1, 128], BF16)
            nc.vector.memset(ones_t[:], 1.0)

            # row 0's replicated pk table ships first on the scalar ring
            # (1 MiB; the ring is otherwise idle until unit 1) so the first
            # mul isn't gated on the on-device PE/ACT broadcast ping-pong;
            # rows 1-7 build on-device during steady state.
            pkrep0_t = pkrepp.tile([128, FD], BF16, name="pkrep0")
            nc.scalar.dma_start(pkrep0_t[:], pkrep0_d[:])

            # small consts follow on the same ring (needed ~10us in)
            pbx = constp.tile([2, NR * NW], BF16)
            nc.scalar.dma_start(pbx[:], pbx_d[:])
            w2t = constp.tile([2, 128], BF16)
            nc.scalar.dma_start(w2t[:], w2_d[:])
            bdt = constp.tile([128, 128], BF16)
            nc.scalar.dma_start(bdt[:], bd_d[:])

            def in_ring(i):
                return nc.sync if i % 2 == 0 else nc.scalar

            def out_ring(i):
                return nc.scalar if i % 2 == 0 else nc.sync

            def build_pkrep(t):
                """Replicate pk row t across all 128 partitions: HWDGE line
                load -> PE ones-matmul -> PSUM -> ACT copy to SBUF."""
                pkrep = pkrepp.tile([128, FD], BF16)
                pkl = pklinep.tile([1, FD], BF16)
                (nc.sync if t % 2 == 0 else nc.scalar).dma_start(pkl[:], pkr_d[t])
                for ch in range(FD // 512):
                    ps = pkpsum.tile([128, 512], F32)
                    nc.tensor.matmul(
                        ps[:],
                        ones_t[:],
                        pkl[:, ch * 512 : (ch + 1) * 512],
                        start=True,
                        stop=True,
                    )
                    nc.scalar.copy(pkrep[:, ch * 512 : (ch + 1) * 512], ps[:])
                return pkrep

            def emit_outmul(st):
                """Deferred modulation+store, three units behind: the unit's
                dead prod buffer becomes the output buffer.  NOTE: 4-D APs
                (leading singleton) — the 3-D form of this broadcast drops
                DVE to 1x mode on HW; the 4-D form (as in v4) keeps 2x."""
                prod, xb, gexp, t, v, i, w0, wlen = st
                o4 = prod.rearrange("p (tl dy q) -> p tl dy q", tl=1, dy=K)
                x4 = xb.rearrange("p (tl dy q) -> p tl dy q", tl=1, dy=K)
                g4 = gexp.rearrange("p (tl a q) -> p tl a q", tl=1, a=1)
                x4b, g4b = bass.broadcast_tensor_aps(x4, g4)
                nc.vector.tensor_tensor(o4, x4b, g4b, op=mybir.AluOpType.mult)
                out_ring(i).dma_start(
                    outr[t, v][:, :, w0 : w0 + wlen],
                    prod.rearrange("p (dy w) -> p dy w", dy=K),
                )

            pkreps = {0: pkrep0_t}

            pend = []

            def emit_unit(i, t, v, w0, wlen):
                xb = xbp.tile([128, K * wlen], BF16)
                in_ring(i).dma_start(
                    xb.rearrange("p (dy w) -> p dy w", dy=K),
                    xr[t, v][:, :, w0 : w0 + wlen],
                )

                # PROD = xb * pkrep[t]  (2x bf16)
                prod = prodp.tile([128, K * wlen], BF16)
                pkq = pkreps[t].rearrange("p (dy w) -> p dy w", dy=K)
                nc.vector.tensor_tensor(
                    prod.rearrange("p (dy w) -> p dy w", dy=K),
                    xb.rearrange("p (dy w) -> p dy w", dy=K),
                    pkq[:, :, w0 : w0 + wlen],
                    op=mybir.AluOpType.mult,
                )

                with nc.allow_low_precision("pairwise bf16 tree adds"):
                    # L1: dy 8 -> 4
                    t1 = t1p.tile([128, K * wlen // 2], BF16)
                    pr4 = prod.rearrange("p (dy q) -> p dy q", dy=K)
                    t14 = t1.rearrange("p (dy q) -> p dy q", dy=K // 2)
                    nc.vector.tensor_tensor(
                        t14,
                        pr4[:, 0 : K // 2, :],
                        pr4[:, K // 2 : K, :],
                        op=mybir.AluOpType.add,
                    )
                    # L2a: dy 4 -> 2
                    t2 = t2p.tile([128, K * wlen // 4], BF16)
                    t14b = t1.rearrange("p (dy q) -> p dy q", dy=4)
                    t24 = t2.rearrange("p (dy q) -> p dy q", dy=2)
                    nc.vector.tensor_tensor(
                        t24,
                        t14b[:, 0:2, :],
                        t14b[:, 2:4, :],
                        op=mybir.AluOpType.add,
                    )
                    # L2b: dy 2 -> 1
                    t3 = t3p.tile([128, wlen], BF16)
                    nc.vector.tensor_tensor(
                        t3.rearrange("p (a q) -> p a q", a=1),
                        t24[:, 0:1, :],
                        t24[:, 1:2, :],
                        op=mybir.AluOpType.add,
                    )

                # g = sum_dx bd.T @ t3[:, :, dx]  +  w2.T @ [pb; 1]
                # (the dx-reduce rides the conv matmul's PSUM accumulation)
                npw = wlen // K
                g = gpsum.tile([128, npw], F32)
                t3x = t3.rearrange("p (q dx) -> p q dx", dx=K)
                for j in range(K):
                    nc.tensor.matmul(
                        g[:],
                        bdt[:],
                        t3x[:, :, j : j + 1],
                        start=(j == 0),
                        stop=False,
                    )
                nc.tensor.matmul(
                    g[:],
                    w2t[:],
                    pbx[:, t * NW + w0 // K : t * NW + w0 // K + npw],
                    start=False,
                    stop=True,
                )

                # modulation of the unit THREE back
                if len(pend) == 3:
                    emit_outmul(pend.pop(0))

                # cast g to bf16 expanded over dx (dense 8-elem runs)
                gexp = gexpp.tile([128, wlen], BF16, tag="gexp")
                ge3 = gexp.rearrange("p (q dx) -> p q dx", dx=K)
                gs3 = g.rearrange("p (q a) -> p q a", a=1)
                ge3b, gs3b = bass.broadcast_tensor_aps(ge3, gs3)
                nc.scalar.copy(ge3b, gs3b)

                pend.append((prod, xb, gexp, t, v, i, w0, wlen))

            units = []
            for i in range(NR * NV):
                t, v = divmod(i, NV)
                units.append((t, v, 0, W))

            for i, (t, v, w0, wlen) in enumerate(units):
                unit_idx = t * NV + v
                # build row t+1's table one row ahead of use
                if unit_idx % NV == 1 and w0 == 0 and t + 1 < NR:
                    pkreps[t + 1] = build_pkrep(t + 1)
                emit_unit(i, t, v, w0, wlen)

            for st in pend:
                emit_outmul(st)

    nc.compile()
    return nc


_PROGRAM = None
LAST_RESULT = None


def make_in_maps(x, pkr, pb, w2, bd):
    import ml_dtypes

    in_maps = []
    for i in range(8):
        r0 = i * NR
        x_core = (
            np.ascontiguousarray(x[:, :, r0 * K : (r0 + NR) * K, :])
            .astype(ml_dtypes.bfloat16)
            .reshape(NV, 128, NR * K, W)
        )
        pkrz = np.ascontiguousarray(pkr[r0 : r0 + NR]).reshape(NR, 1, FD)
        pkrep0 = np.ascontiguousarray(np.broadcast_to(pkr[r0], (128, FD)))
        pbx = np.empty((2, NR * NW), np.float32)
        pbx[0] = pb[r0 * NW : (r0 + NR) * NW]
        pbx[1] = 1.0
        pbx = pbx.astype(ml_dtypes.bfloat16)
        in_maps.append(
            {
                "x": x_core,
                "pkr": pkrz,
                "pkrep0": pkrep0,
                "pbx": pbx,
                "w2": w2.astype(ml_dtypes.bfloat16),
                "bd": bd.astype(ml_dtypes.bfloat16),
            }
        )
    return in_maps


def kernel(**inputs):
    global _PROGRAM, LAST_RESULT
    x = np.ascontiguousarray(np.asarray(inputs["x"], dtype=np.float32))
    pkr, pb, w2, bd, pk = _host_tables(
        *[
            np.asarray(inputs[k], dtype=np.float32)
            for k in (
                "w1k", "b1k", "w2k", "b2k",
                "w1b", "b1b", "w2b", "b2b",
                "conv_w", "conv_b",
            )
        ]
    )
    if _PROGRAM is None:
        _PROGRAM = build_program()
    nc = _PROGRAM

    in_maps = make_in_maps(x, pkr, pb, w2, bd)

    conv_w = np.asarray(inputs["conv_w"], np.float64)
    conv_b = np.asarray(inputs["conv_b"], np.float64)

    def _spot_check(out):
        """Verify a sample of patches against the exact host formula;
        catches the rare silent device corruption (bf16 path ~0.4%/elem)."""
        rng = np.random.default_rng(1234)
        worst = 0.0
        for _ in range(32):
            b = int(rng.integers(B))
            ph = int(rng.integers(NH))
            pw = int(rng.integers(NW))
            l = ph * NW + pw
            patch = x[b, :, ph * K : (ph + 1) * K, pw * K : (pw + 1) * K]
            patch = patch.reshape(C, K * K).astype(np.float64)
            feats = patch @ pk[l] + pb[l]
            g = conv_w @ feats + conv_b
            exp = patch * g[:, None]
            got = out[b, :, ph * K : (ph + 1) * K, pw * K : (pw + 1) * K]
            got = got.reshape(C, K * K).astype(np.float64)
            denom = np.linalg.norm(exp) + 1e-30
            worst = max(worst, float(np.linalg.norm(got - exp) / denom))
        return worst

    res = None
    for attempt in range(4):
        try:
            res = run_bass_kernel_spmd(nc, in_maps, list(range(8)))
        except Exception:
            if attempt == 3:
                raise
            continue
        out = np.empty((B, C, H, W), np.float32)
        for i in range(8):
            r0 = i * NR
            out[:, :, r0 * K : (r0 + NR) * K, :] = (
                res.results[i]["out"].astype(np.float32).reshape(B, C, NR * K, W)
            )
        err = _spot_check(out)
        if err < 0.05:
            break
        if attempt == 3:
            raise RuntimeError(f"device output failed spot check ({err:.3f})")
    LAST_RESULT = res
    return out
